# revision 44
# baseline (speedup 1.0000x reference)
"""Trainium2 Bass kernel for causal self-attention with cumulative-phase rotary
embedding (nn_CausalSelfAttention_64338610094602).

Sharding: 8 cores = 4 batches x 2 head-groups (tensor-parallel over heads).
Each core computes, for its (batch, 8-head group):
  omega/phi (replicated per batch), QKV projections, rotation + RMSNorm,
  causal attention (transposed-scores layout, max-free softmax), and a
  partial output projection. Host sums the two head-group partials per batch.

v4 design notes:
  - All big GEMM operands in bf16 (matmul rate keys off the moving operand;
    bf16 is 1 cycle/row at any N). PSUM accumulation stays fp32.
  - PSUM: one [128,512] ring of 4 banks (tag "q") shared by projections, v,
    scores, and P3 chains; [128,512] yps ring 2; [1,512] ring 2 for
    omega/ssq/denominators. Every accumulation chain owns a full bank
    (matmul start=True zeroes the whole bank).
  - Rotation: gamma folded into trig tiles (negated second half) so one
    full-width multiply + two swapped-half multiplies + one add write
    q_sb/k_sb directly in bf16.
  - RMSNorm: ACT Square -> M=1 PE colsum (deferred one site so the PE never
    waits on ACT) -> one Abs_reciprocal_sqrt -> GpSimd partition_broadcast.
    Same ACT table as Square/Exp path: no table swaps in steady state.
  - Causal mask folded into the PE accumulation: for diagonal score tiles,
    one extra matmul (tril stationary x -1e9 step moving) adds -1e9*count
    to masked entries, so exp() gives exact zeros.
  - 2c software-pipelined with lookahead-2 at key-tile granularity: scores
    for I and I-1 are issued before the consumers of I-2, giving the PE
    ~2us of cover work over the ACT Exp latency.
  - Softmax denominator reciprocal broadcast via GpSimd partition_broadcast;
    y spilled to DRAM in bf16 and streamed back in P3 (cb-outer loop with
    wo column-block and y tile prefetch).
"""
import math

import numpy as np
import ml_dtypes

import concourse.mybir as mybir
import concourse.tile as tile
from concourse import bacc
from concourse.bass_utils import run_bass_kernel_spmd

B, T, C = 4, 2048, 2048
H, D, DH = 16, 128, 64
HG = 8          # heads per core (head-group)
GD = HG * D     # group output dims = 1024
NT = T // 512   # 4 query blocks of 512
NCT = C // 128  # 16 contraction tiles
EPS = 1e-5
SCL = 1.0 / math.sqrt(D)
NEG = -1.0e9

dt = mybir.dt
AF = mybir.ActivationFunctionType
ALU = mybir.AluOpType

TWO_PI = 6.283185307179586
INV_2PI = 1.0 / TWO_PI
CW1 = float(np.float32(6.28125))
CW2 = float(np.float32(TWO_PI - 6.28125))
CW3 = float(TWO_PI - CW1 - float(np.float32(TWO_PI - 6.28125)))
MAGIC = 12582912.0  # 1.5 * 2^23: fp32 add/sub rounds to nearest int
HALF_PI = 1.5707963267948966
PI = 3.141592653589793

_CACHE = {}
DEBUG = False


def _build():
    f32, bf16 = dt.float32, dt.bfloat16
    nc = bacc.Bacc(None, target_bir_lowering=False)
    with tile.TileContext(nc) as tc:
        xt_d = nc.dram_tensor("xt", (C, T), bf16, kind="ExternalInput")
        wq_d = nc.dram_tensor("wq", (C, GD), bf16, kind="ExternalInput")
        wk_d = nc.dram_tensor("wk", (C, GD), bf16, kind="ExternalInput")
        wv_d = nc.dram_tensor("wv", (C, GD), bf16, kind="ExternalInput")
        wo_d = nc.dram_tensor("wo", (GD, C), bf16, kind="ExternalInput")
        womg_d = nc.dram_tensor("womg", (128, NCT), bf16, kind="ExternalInput")
        b16_d = nc.dram_tensor("b16", (1, 1), f32, kind="ExternalInput")
        logf2_d = nc.dram_tensor("logf2", (128, 1), f32, kind="ExternalInput")
        gq_d = nc.dram_tensor("gq", (128, 1), f32, kind="ExternalInput")
        gqB_d = nc.dram_tensor("gqB", (128, 1), f32, kind="ExternalInput")
        gk_d = nc.dram_tensor("gk", (128, 1), f32, kind="ExternalInput")
        gkB_d = nc.dram_tensor("gkB", (128, 1), f32, kind="ExternalInput")
        trilA_d = nc.dram_tensor("trilA", (128, 128), bf16, kind="ExternalInput")
        maskB_d = nc.dram_tensor("maskB", (128, 4 * 512), bf16, kind="ExternalInput")
        onesA_d = nc.dram_tensor("onesA", (128, 1), bf16, kind="ExternalInput")
        out_d = nc.dram_tensor("out", (T, C), f32, kind="ExternalOutput")
        dbg = {}
        if DEBUG:
            dbg["q"] = nc.dram_tensor("dbg_q", (128, 2 * T), bf16,
                                      kind="ExternalOutput")
            dbg["k"] = nc.dram_tensor("dbg_k", (128, 2 * T), bf16,
                                      kind="ExternalOutput")
            dbg["v"] = nc.dram_tensor("dbg_v", (128, 16 * 256), bf16,
                                      kind="ExternalOutput")
            dbg["y"] = nc.dram_tensor("dbg_y", (128, HG * T), bf16,
                                      kind="ExternalOutput")

        with tc.tile_pool(name="const", bufs=1) as constp, \
             tc.tile_pool(name="dram", bufs=1, space="DRAM") as dramp, \
             tc.tile_pool(name="psp", bufs=1, space="PSUM") as psp:

            # ---- constants ----
            womg = constp.tile([128, NCT], bf16)
            nc.sync.dma_start(womg[:], womg_d[:])
            b16t = constp.tile([1, 1], f32)
            nc.sync.dma_start(b16t[:], b16_d[:])
            logf2 = constp.tile([128, 1], f32)
            nc.sync.dma_start(logf2[:], logf2_d[:])
            gq = constp.tile([128, 1], f32)
            nc.sync.dma_start(gq[:], gq_d[:])
            gqB = constp.tile([128, 1], f32)
            nc.sync.dma_start(gqB[:], gqB_d[:])
            gk = constp.tile([128, 1], f32)
            nc.sync.dma_start(gk[:], gk_d[:])
            gkB = constp.tile([128, 1], f32)
            nc.sync.dma_start(gkB[:], gkB_d[:])
            trilA = constp.tile([128, 128], bf16)
            nc.sync.dma_start(trilA[:], trilA_d[:])
            maskB = constp.tile([128, 4 * 512], bf16)
            nc.sync.dma_start(maskB[:], maskB_d[:])
            onesA = constp.tile([128, 1], bf16)
            nc.sync.dma_start(onesA[:], onesA_d[:])
            freq2 = constp.tile([128, 1], f32)
            nc.scalar.activation(freq2[:], logf2[:], AF.Exp)
            eps128 = constp.tile([128, 1], f32)
            nc.vector.memset(eps128[:], EPS)

            y_d = dramp.tile([128, HG * T], bf16)  # yT per head at col h*T

            with tc.tile_pool(name="big", bufs=1) as bigp, \
                 tc.tile_pool(name="xtp", bufs=1) as xtp, \
                 tc.tile_pool(name="wstp", bufs=1) as wstp, \
                 tc.tile_pool(name="scp", bufs=1) as scp, \
                 tc.tile_pool(name="rowp", bufs=1) as rowp:
                q_sb = bigp.tile([128, 2 * T], bf16)   # [D, hl*T + t]
                k_sb = bigp.tile([128, 2 * T], bf16)
                v_sb = bigp.tile([128, 16 * 256], bf16)  # key tile tt at tt*256
                trigA_q = bigp.tile([128, T], bf16)
                trigB_q = bigp.tile([128, T], bf16)
                trigA_k = bigp.tile([128, T], bf16)
                trigB_k = bigp.tile([128, T], bf16)
                _main(nc, tc, xt_d, wq_d, wk_d, wv_d,
                      xtp, wstp, scp, rowp, psp,
                      womg, b16t, freq2, gq, gqB, gk, gkB, trilA, maskB,
                      onesA, eps128, q_sb, k_sb, v_sb, y_d,
                      trigA_q, trigB_q, trigA_k, trigB_k, dbg)

            # ---- P3: out = y^T W_o (partial over heads); cb-outer with
            # wo column-block + y tile streaming ----
            with tc.tile_pool(name="p3w", bufs=1) as p3w, \
                 tc.tile_pool(name="p3o", bufs=1) as p3o:
                wo_slots = [None, None]
                y_slots = [None] * 4

                def issue_wo(cb):
                    wob = p3w.tile([128, HG * 512], bf16, tag="wo", bufs=2,
                                   name=f"wo_{cb}")
                    for hh in range(HG):
                        nc.sync.dma_start(
                            wob[:, hh * 512:(hh + 1) * 512],
                            wo_d[hh * 128:(hh + 1) * 128,
                                 cb * 512:(cb + 1) * 512])
                    wo_slots[cb % 2] = wob

                def issue_y(cb, ti):
                    yti = p3w.tile([128, HG * 128], bf16, tag="yt", bufs=4,
                                   name=f"yti_{cb}_{ti}")
                    for hh in range(HG):
                        nc.sync.dma_start(
                            yti[:, hh * 128:(hh + 1) * 128],
                            y_d[:, hh * T + ti * 128:hh * T + (ti + 1) * 128])
                    y_slots[ti % 4] = yti

                NTI = T // 128
                issue_wo(0)
                for ti in range(3):
                    issue_y(0, ti)
                for cb in range(C // 512):
                    if cb + 1 < C // 512:
                        issue_wo(cb + 1)
                    for ti in range(NTI):
                        nti = ti + 3
                        if nti >= NTI:
                            if cb + 1 < C // 512:
                                issue_y(cb + 1, nti - NTI)
                        else:
                            issue_y(cb, nti)
                        yti = y_slots[ti % 4]
                        wob = wo_slots[cb % 2]
                        ops2 = psp.tile([128, 1024], f32, tag="s", bufs=2,
                                        name=f"ops_{cb}_{ti}")
                        ops = ops2[:, 0:512]
                        # (slice AP used directly below)
                        for hh in range(HG):
                            nc.tensor.matmul(
                                ops,
                                yti[:, hh * 128:(hh + 1) * 128],
                                wob[:, hh * 512:(hh + 1) * 512],
                                start=(hh == 0), stop=(hh == HG - 1))
                        osb = p3o.tile([128, 512], f32, tag="osb", bufs=3)
                        nc.scalar.copy(osb[:], ops)
                        nc.sync.dma_start(
                            out_d[ti * 128:(ti + 1) * 128,
                                  cb * 512:(cb + 1) * 512],
                            osb[:])
    nc.compile()
    return nc


def _main(nc, tc, xt_d, wq_d, wk_d, wv_d,
          xtp, wstp, scp, rowp, psp,
          womg, b16t, freq2, gq, gqB, gk, gkB, trilA, maskB,
          onesA, eps128, q_sb, k_sb, v_sb, y_d,
          trigA_q, trigB_q, trigA_k, trigB_k, dbg):
    f32, bf16 = dt.float32, dt.bfloat16

    # x^T tiles, quarter-major DMA order so P1/2a can start early
    xts = xtp.tile([128, NCT * T], bf16)  # c-tile i at cols [i*T,(i+1)*T)
    for quarter in range(4):
        for i in range(NCT):
            cs = quarter * 512
            nc.sync.dma_start(
                xts[:, i * T + cs:i * T + cs + 512],
                xt_d[i * 128:(i + 1) * 128, cs:cs + 512])

    # ---- P1: omega -> phi -> trig (PE does only the omega matvecs) ----
    with tc.tile_pool(name="p1p", bufs=1) as p1p:
        omega = rowp.tile([1, T], f32, tag="om")
        for J in range(NT):
            omps = psp.tile([1, 512], f32, tag="d", bufs=2, name=f"omps_{J}")
            for i in range(NCT):
                nc.tensor.matmul(
                    omps[:], womg[:, i:i + 1],
                    xts[:, i * T + J * 512:i * T + J * 512 + 512],
                    start=(i == 0), stop=(i == NCT - 1))
            nc.scalar.activation(omega[:, J * 512:(J + 1) * 512], omps[:],
                                 AF.Sigmoid, scale=1.0 / 16.0, bias=b16t[:])
        incl = rowp.tile([1, T], f32, tag="incl")
        nc.vector.tensor_tensor_scan(incl[:], omega[:], omega[:], 0.0,
                                     ALU.add, ALU.bypass)
        phi = rowp.tile([1, T], f32, tag="phi")
        nc.vector.tensor_sub(phi[:], incl[:], omega[:])
        for J in range(NT):
            sl = slice(J * 512, (J + 1) * 512)
            phi2 = p1p.tile([128, 512], f32, tag="p1", bufs=4,
                            name=f"phi2_{J}")
            nc.gpsimd.partition_broadcast(phi2[:], phi[:, sl])
            ang = p1p.tile([128, 512], f32, tag="p1", bufs=4, name=f"ang_{J}")
            nc.vector.tensor_scalar(ang[:], phi2[:], freq2[:], None,
                                    op0=ALU.mult)
            mm = p1p.tile([128, 512], f32, tag="p1", bufs=4, name=f"mm_{J}")
            nc.vector.tensor_scalar(mm[:], ang[:], INV_2PI, MAGIC,
                                    op0=ALU.mult, op1=ALU.add)
            kk = p1p.tile([128, 512], f32, tag="p1", bufs=4, name=f"kk_{J}")
            nc.vector.tensor_scalar_add(kk[:], mm[:], -MAGIC)
            red = p1p.tile([128, 512], f32, tag="p1", bufs=4, name=f"red_{J}")
            nc.vector.cody_waite_cascade(red[:], ang[:], kk[:], CW1, CW2, CW3)
            red2 = p1p.tile([128, 512], f32, tag="p1", bufs=4,
                            name=f"red2_{J}")
            nc.vector.add_range_wrap(red2[:], red[:], HALF_PI, PI, TWO_PI)
            sinr = p1p.tile([128, 512], f32, tag="p1", bufs=4,
                            name=f"sinr_{J}")
            nc.scalar.activation(sinr[:], red[:], AF.Sin)
            cosr = p1p.tile([128, 512], f32, tag="p1", bufs=4,
                            name=f"cosr_{J}")
            nc.scalar.activation(cosr[:], red2[:], AF.Sin)
            nc.scalar.activation(trigA_q[:, sl], cosr[:], AF.Copy, scale=gq[:])
            nc.scalar.activation(trigB_q[:, sl], sinr[:], AF.Copy, scale=gqB[:])
            nc.scalar.activation(trigA_k[:, sl], cosr[:], AF.Copy, scale=gk[:])
            nc.scalar.activation(trigB_k[:, sl], sinr[:], AF.Copy, scale=gkB[:])

    # ---- P2 per pair ----
    pend_norm = [None]
    pend_epi = [None]
    ssq_queue = []  # FIFO of deferred rstd tails, emitted one site later

    def flush(pend):
        if pend[0] is not None:
            pend[0]()
            pend[0] = None

    sites = [(pair, wi, hl) for pair in range(4) for wi in range(2)
             for hl in range(2)]
    wp_slots = [None, None]

    def issue_panel(si):
        pair, wi, hl = sites[si]
        h = pair * 2 + hl
        w_d = (wq_d, wk_d)[wi]
        wp = wstp.tile([128, NCT * 128], bf16, tag="wp", bufs=2,
                       name=f"wp_{si}")
        for i in range(NCT):
            nc.sync.dma_start(
                wp[:, i * 128:(i + 1) * 128],
                w_d[i * 128:(i + 1) * 128, h * 128:(h + 1) * 128])
        wp_slots[si % 2] = wp

    issue_panel(0)

    for pair in range(4):
        # wv panel for this pair (resident; streamed during 2a)
        wvp = wstp.tile([128, NCT * 256], bf16, tag="wvp", bufs=1,
                        name=f"wvp_{pair}")
        for i in range(NCT):
            nc.sync.dma_start(
                wvp[:, i * 256:(i + 1) * 256],
                wv_d[i * 128:(i + 1) * 128, pair * 256:(pair + 1) * 256])

        # --- 2a: q/k for both heads ---
        for wi in range(2):
            for hl in range(2):
                si = pair * 4 + wi * 2 + hl
                if si + 1 < len(sites):
                    issue_panel(si + 1)
                wp = wp_slots[si % 2]
                trigA = (trigA_q, trigA_k)[wi]
                trigB = (trigB_q, trigB_k)[wi]
                dest = (q_sb, k_sb)[wi]
                rnbs = []
                sqs = []
                for Jp in range(2):
                    # deferred rstd tails (one-site lag, one J-group per
                    # flush so the [1,512] psum ring never backs up)
                    while len(ssq_queue) > 1:
                        ssq_queue.pop(0)()
                    qps2 = psp.tile([128, 1024], f32, tag="s", bufs=2,
                                    name=f"qps2_{si}_{Jp}")
                    for i in range(NCT):
                        for Jh in range(2):
                            J = 2 * Jp + Jh
                            nc.tensor.matmul(
                                qps2[:, Jh * 512:(Jh + 1) * 512],
                                wp[:, i * 128:(i + 1) * 128],
                                xts[:, i * T + J * 512:i * T + J * 512 + 512],
                                start=(i == 0), stop=(i == NCT - 1))
                    for Jh in range(2):
                        J = 2 * Jp + Jh
                        qps = qps2[:, Jh * 512:(Jh + 1) * 512]
                        sl = slice(J * 512, (J + 1) * 512)
                        dcol = hl * T + J * 512
                        # rotation: A + swapped-half B, gamma folded in trig
                        A = scp.tile([128, 512], f32, tag="ra", bufs=2,
                                     name=f"A_{si}_{J}")
                        nc.vector.tensor_tensor(A[:], qps, trigA[:, sl],
                                                op=ALU.mult)
                        Bt = scp.tile([128, 512], f32, tag="rb", bufs=2,
                                      name=f"Bt_{si}_{J}")
                        nc.vector.tensor_tensor(
                            Bt[0:DH, :],
                            qps2[DH:128, Jh * 512:(Jh + 1) * 512],
                            trigB[0:DH, sl], op=ALU.mult)
                        nc.vector.tensor_tensor(
                            Bt[DH:128, :],
                            qps2[0:DH, Jh * 512:(Jh + 1) * 512],
                            trigB[DH:128, sl], op=ALU.mult)
                        nc.vector.tensor_add(
                            dest[:, dcol:dcol + 512], A[:], Bt[:])
                        # sum-of-squares path (rotation preserves norms)
                        sq = scp.tile([128, 512], bf16, tag="sq", bufs=4,
                                      name=f"sq_{si}_{J}")
                        nc.scalar.activation(sq[:], qps, AF.Square)
                        sqs.append((J, sq))

                    def ssq_tail(sqs=tuple(sqs[-2:]), rnbs=rnbs, si=si):
                        for J, sq in sqs:
                            ssqps = psp.tile([1, 512], f32, tag="d", bufs=2,
                                             name=f"ssq_{si}_{J}")
                            nc.tensor.matmul(ssqps[:], onesA[:], sq[:],
                                             start=True, stop=True)
                            rrow = rowp.tile([1, 512], bf16, tag="rr", bufs=2,
                                             name=f"rrow_{si}_{J}")
                            nc.scalar.activation(rrow[:], ssqps[:],
                                                 AF.Abs_reciprocal_sqrt,
                                                 scale=1.0 / 128.0,
                                                 bias=eps128[0:1, :])
                            rnb = scp.tile([128, 512], bf16, tag="rnb",
                                           bufs=4, name=f"rnb_{si}_{J}")
                            nc.gpsimd.partition_broadcast(rnb[:], rrow[:])
                            rnbs.append((J, rnb))
                    ssq_queue.append(ssq_tail)
                flush(pend_norm)

                def norm(dest=dest, hl=hl, rnbs=rnbs):
                    for J, rnb in rnbs:
                        dcol = hl * T + J * 512
                        nc.vector.tensor_tensor(
                            dest[:, dcol:dcol + 512],
                            dest[:, dcol:dcol + 512],
                            rnb[:], op=ALU.mult)
                pend_norm[0] = norm
        while ssq_queue:
            ssq_queue.pop(0)()
        flush(pend_norm)
        if dbg and pair == 0:
            nc.sync.dma_start(dbg["q"][:], q_sb[:])
            nc.sync.dma_start(dbg["k"][:], k_sb[:])

        # --- 2b: v for both heads; each accumulation chain owns a full PSUM
        # bank: quarters 0 and 2 of two [128,1024] tiles = 4 banks ---
        for tq in range(4):
            vps = []
            for q4 in range(2):
                vps.append(psp.tile([128, 1024], f32, tag="s", bufs=2,
                                    name=f"vps_{pair}_{tq}_{q4}"))
            for i in range(NCT):
                for t in range(4):
                    tt = tq * 4 + t
                    nc.tensor.matmul(
                        vps[t // 2][:, (t % 2) * 512:(t % 2) * 512 + 256],
                        xts[:, i * T + tt * 128:i * T + (tt + 1) * 128],
                        wvp[:, i * 256:(i + 1) * 256],
                        start=(i == 0), stop=(i == NCT - 1))
            for t in range(4):
                tt = tq * 4 + t
                nc.vector.tensor_copy(
                    v_sb[:, tt * 256:(tt + 1) * 256],
                    vps[t // 2][:, (t % 2) * 512:(t % 2) * 512 + 256])
        if dbg and pair == 0:
            nc.sync.dma_start(dbg["v"][:], v_sb[:])

        # --- 2c: attention, software-pipelined lookahead-2 ---
        for hl in range(2):
            h = pair * 2 + hl
            for J in range(NT):
                nI = 4 * J + 4
                yps = psp.tile([128, 512], f32, tag="y", bufs=2,
                               name=f"yps_{pair}_{hl}_{J}")
                dps = psp.tile([1, 512], f32, tag="d", bufs=2,
                               name=f"dps_{pair}_{hl}_{J}")
                exq = []

                def consume(ex2, I0, yps=yps, dps=dps, hl=hl, nI=nI):
                    for half2 in range(2):
                        I = I0 + half2
                        exsl = ex2[:, half2 * 512:(half2 + 1) * 512]
                        nc.tensor.matmul(
                            yps[:],
                            v_sb[:, I * 256 + hl * 128:I * 256 + hl * 128 + 128],
                            exsl, start=(I == 0), stop=(I == nI - 1))
                        nc.tensor.matmul(
                            dps[:], onesA[:], exsl,
                            start=(I == 0), stop=(I == nI - 1))

                for Ip in range(nI // 2):
                    sps2 = psp.tile([128, 1024], f32, tag="s", bufs=2,
                                    name=f"sps_{pair}_{hl}_{J}_{Ip}")
                    for half2 in range(2):
                        I = 2 * Ip + half2
                        diag = I >= 4 * J
                        osl = sps2[:, half2 * 512:(half2 + 1) * 512]
                        nc.tensor.matmul(
                            osl,
                            k_sb[:, hl * T + I * 128:hl * T + (I + 1) * 128],
                            q_sb[:, hl * T + J * 512:hl * T + (J + 1) * 512],
                            start=True, stop=(not diag))
                        if diag:
                            r = I - 4 * J
                            nc.tensor.matmul(
                                osl, trilA[:], maskB[:, r * 512:(r + 1) * 512],
                                start=False, stop=True)
                    ex2 = scp.tile([128, 1024], bf16, tag="ex", bufs=3,
                                   name=f"ex_{pair}_{hl}_{J}_{Ip}")
                    nc.scalar.activation(ex2[:], sps2[:], AF.Exp, scale=SCL)
                    exq.append((ex2, 2 * Ip))
                    if len(exq) > 2:
                        consume(*exq.pop(0))
                    if Ip == 0:
                        flush(pend_epi)
                while exq:
                    consume(*exq.pop(0))

                def epilogue(yps=yps, dps=dps, h=h, J=J):
                    rowt = rowp.tile([1, 512], f32, tag="rc", bufs=2,
                                     name=f"rc_{h}_{J}")
                    nc.vector.reciprocal_approx_fast(out=rowt[:], in_=dps[:])
                    rb = scp.tile([128, 512], f32, tag="rbc", bufs=2,
                                  name=f"rb_{h}_{J}")
                    nc.gpsimd.partition_broadcast(rb[:], rowt[:])
                    yt = scp.tile([128, 512], bf16, tag="yt", bufs=2,
                                  name=f"yt_{h}_{J}")
                    nc.vector.tensor_tensor(yt[:], yps[:], rb[:], op=ALU.mult)
                    nc.sync.dma_start(
                        y_d[:, h * T + J * 512:h * T + (J + 1) * 512], yt[:])
                    if dbg:
                        nc.sync.dma_start(
                            dbg["y"][:, h * T + J * 512:h * T + (J + 1) * 512],
                            yt[:])
                pend_epi[0] = epilogue
        flush(pend_epi)


def _host_prep(inputs):
    bf = ml_dtypes.bfloat16
    x = np.asarray(inputs["x"], dtype=np.float32)
    Wq = np.asarray(inputs["Wq"], dtype=np.float32)
    Wk = np.asarray(inputs["Wk"], dtype=np.float32)
    Wv = np.asarray(inputs["Wv"], dtype=np.float32)
    Wo = np.asarray(inputs["Wo"], dtype=np.float32)
    w_omega = np.asarray(inputs["w_omega"], dtype=np.float32)
    b_omega = np.asarray(inputs["b_omega"], dtype=np.float32)
    log_freq = np.asarray(inputs["log_freq"], dtype=np.float32)
    q_gamma = np.asarray(inputs["q_gamma"], dtype=np.float32)
    k_gamma = np.asarray(inputs["k_gamma"], dtype=np.float32)

    womg = w_omega.reshape(NCT, 128).T.astype(bf)  # [p, i] = w_omega[i*128+p]
    b16 = (b_omega / 16.0).reshape(1, 1).astype(np.float32)
    logf2 = np.concatenate([log_freq, log_freq]).reshape(128, 1)
    gqv = q_gamma.reshape(128, 1).astype(np.float32)
    gqB = np.concatenate([q_gamma[:DH], -q_gamma[DH:]]).reshape(128, 1)
    gkv = k_gamma.reshape(128, 1).astype(np.float32)
    gkB = np.concatenate([k_gamma[:DH], -k_gamma[DH:]]).reshape(128, 1)
    kk = np.arange(128)
    trilA = (kk[:, None] <= kk[None, :]).astype(bf)  # [k, p] = (k <= p)
    p = np.arange(128)[:, None]
    c = np.arange(512)[None, :]
    maskB = np.concatenate(
        [(NEG * ((p + r * 128) > c)).astype(np.float32) for r in range(4)],
        axis=1).astype(bf)
    onesA = np.ones((128, 1), dtype=bf)

    in_maps = []
    for core in range(8):
        b, g = core // 2, core % 2
        in_maps.append({
            "xt": np.ascontiguousarray(x[b].T).astype(bf),
            "wq": np.ascontiguousarray(Wq[g * GD:(g + 1) * GD, :].T).astype(bf),
            "wk": np.ascontiguousarray(Wk[g * GD:(g + 1) * GD, :].T).astype(bf),
            "wv": np.ascontiguousarray(Wv[g * GD:(g + 1) * GD, :].T).astype(bf),
            "wo": np.ascontiguousarray(Wo[:, g * GD:(g + 1) * GD].T).astype(bf),
            "womg": womg, "b16": b16,
            "logf2": logf2.astype(np.float32),
            "gq": gqv, "gqB": gqB.astype(np.float32),
            "gk": gkv, "gkB": gkB.astype(np.float32),
            "trilA": trilA, "maskB": maskB, "onesA": onesA,
        })
    return in_maps


def kernel(**inputs) -> np.ndarray:
    if "nc" not in _CACHE:
        _CACHE["nc"] = _build()
    nc = _CACHE["nc"]
    in_maps = _host_prep(inputs)
    res = run_bass_kernel_spmd(nc, in_maps, core_ids=list(range(8)))
    out = np.empty((B, T, C), dtype=np.float32)
    for b in range(B):
        out[b] = res.results[2 * b]["out"] + res.results[2 * b + 1]["out"]
    return out


# revision 48
# speedup vs baseline: 1.4143x; 1.4143x over previous
"""Trainium2 Bass kernel for causal self-attention with cumulative-phase rotary
embedding (nn_CausalSelfAttention_64338610094602).

Sharding: 8 cores = 4 batches x 2 head-groups (tensor-parallel over heads).
Each core computes, for its (batch, 8-head group):
  omega/phi (replicated per batch), QKV projections, rotation + RMSNorm,
  causal attention (transposed-scores layout, max-free softmax), and a
  partial output projection. Host sums the two head-group partials per batch.

v4 design notes:
  - All big GEMM operands in bf16 (matmul rate keys off the moving operand;
    bf16 is 1 cycle/row at any N). PSUM accumulation stays fp32.
  - PSUM: one [128,512] ring of 4 banks (tag "q") shared by projections, v,
    scores, and P3 chains; [128,512] yps ring 2; [1,512] ring 2 for
    omega/ssq/denominators. Every accumulation chain owns a full bank
    (matmul start=True zeroes the whole bank).
  - Rotation: gamma folded into trig tiles (negated second half) so one
    full-width multiply + two swapped-half multiplies + one add write
    q_sb/k_sb directly in bf16.
  - RMSNorm: ACT Square -> M=1 PE colsum (deferred one site so the PE never
    waits on ACT) -> one Abs_reciprocal_sqrt -> GpSimd partition_broadcast.
    Same ACT table as Square/Exp path: no table swaps in steady state.
  - Causal mask folded into the PE accumulation: for diagonal score tiles,
    one extra matmul (tril stationary x -1e9 step moving) adds -1e9*count
    to masked entries, so exp() gives exact zeros.
  - 2c software-pipelined with lookahead-2 at key-tile granularity: scores
    for I and I-1 are issued before the consumers of I-2, giving the PE
    ~2us of cover work over the ACT Exp latency.
  - Softmax denominator reciprocal broadcast via GpSimd partition_broadcast;
    y spilled to DRAM in bf16 and streamed back in P3 (cb-outer loop with
    wo column-block and y tile prefetch).
"""
import math

import numpy as np
import ml_dtypes

import concourse.mybir as mybir
import concourse.tile as tile
from concourse import bacc
from concourse.bass_utils import run_bass_kernel_spmd

B, T, C = 4, 2048, 2048
H, D, DH = 16, 128, 64
HG = 8          # heads per core (head-group)
GD = HG * D     # group output dims = 1024
NT = T // 512   # 4 query blocks of 512
NCT = C // 128  # 16 contraction tiles
EPS = 1e-5
SCL = 1.0 / math.sqrt(D)
NEG = -1.0e9

dt = mybir.dt
AF = mybir.ActivationFunctionType
ALU = mybir.AluOpType

TWO_PI = 6.283185307179586
INV_2PI = 1.0 / TWO_PI
CW1 = float(np.float32(6.28125))
CW2 = float(np.float32(TWO_PI - 6.28125))
CW3 = float(TWO_PI - CW1 - float(np.float32(TWO_PI - 6.28125)))
MAGIC = 12582912.0  # 1.5 * 2^23: fp32 add/sub rounds to nearest int
HALF_PI = 1.5707963267948966
PI = 3.141592653589793

_CACHE = {}
DEBUG = False


def _build():
    f32, bf16 = dt.float32, dt.bfloat16
    nc = bacc.Bacc(None, target_bir_lowering=False)
    with tile.TileContext(nc) as tc:
        xt_d = nc.dram_tensor("xt", (C, T), bf16, kind="ExternalInput")
        wq_d = nc.dram_tensor("wq", (C, GD), bf16, kind="ExternalInput")
        wk_d = nc.dram_tensor("wk", (C, GD), bf16, kind="ExternalInput")
        wv_d = nc.dram_tensor("wv", (C, GD), bf16, kind="ExternalInput")
        wo_d = nc.dram_tensor("wo", (GD, C), bf16, kind="ExternalInput")
        womg2_d = nc.dram_tensor("womg2", (128, NCT * 128), bf16,
                                 kind="ExternalInput")
        b16_d = nc.dram_tensor("b16", (1, 1), f32, kind="ExternalInput")
        logf2_d = nc.dram_tensor("logf2", (128, 1), f32, kind="ExternalInput")
        gq_d = nc.dram_tensor("gq", (128, 1), f32, kind="ExternalInput")
        gqB_d = nc.dram_tensor("gqB", (128, 1), f32, kind="ExternalInput")
        gk_d = nc.dram_tensor("gk", (128, 1), f32, kind="ExternalInput")
        gkB_d = nc.dram_tensor("gkB", (128, 1), f32, kind="ExternalInput")
        trilA_d = nc.dram_tensor("trilA", (128, 128), bf16, kind="ExternalInput")
        maskB_d = nc.dram_tensor("maskB", (128, 4 * 512), bf16, kind="ExternalInput")
        ones128_d = nc.dram_tensor("ones128", (128, 128), bf16,
                                   kind="ExternalInput")
        out_d = nc.dram_tensor("out", (T, C), f32, kind="ExternalOutput")
        dbg = {}
        if DEBUG:
            dbg["q"] = nc.dram_tensor("dbg_q", (128, 2 * T), bf16,
                                      kind="ExternalOutput")
            dbg["k"] = nc.dram_tensor("dbg_k", (128, 2 * T), bf16,
                                      kind="ExternalOutput")
            dbg["v"] = nc.dram_tensor("dbg_v", (128, 16 * 256), bf16,
                                      kind="ExternalOutput")
            dbg["y"] = nc.dram_tensor("dbg_y", (128, HG * T), bf16,
                                      kind="ExternalOutput")

        with tc.tile_pool(name="const", bufs=1) as constp, \
             tc.tile_pool(name="dram", bufs=1, space="DRAM") as dramp, \
             tc.tile_pool(name="psp", bufs=1, space="PSUM") as psp:

            # ---- constants ----
            womg2 = constp.tile([128, NCT * 128], bf16)
            nc.sync.dma_start(womg2[:], womg2_d[:])
            b16t = constp.tile([1, 1], f32)
            nc.sync.dma_start(b16t[:], b16_d[:])
            logf2 = constp.tile([128, 1], f32)
            nc.sync.dma_start(logf2[:], logf2_d[:])
            gq = constp.tile([128, 1], f32)
            nc.sync.dma_start(gq[:], gq_d[:])
            gqB = constp.tile([128, 1], f32)
            nc.sync.dma_start(gqB[:], gqB_d[:])
            gk = constp.tile([128, 1], f32)
            nc.sync.dma_start(gk[:], gk_d[:])
            gkB = constp.tile([128, 1], f32)
            nc.sync.dma_start(gkB[:], gkB_d[:])
            trilA = constp.tile([128, 128], bf16)
            nc.sync.dma_start(trilA[:], trilA_d[:])
            maskB = constp.tile([128, 4 * 512], bf16)
            nc.sync.dma_start(maskB[:], maskB_d[:])
            ones128 = constp.tile([128, 128], bf16)
            nc.sync.dma_start(ones128[:], ones128_d[:])
            freq2 = constp.tile([128, 1], f32)
            nc.scalar.activation(freq2[:], logf2[:], AF.Exp)
            eps128 = constp.tile([128, 1], f32)
            nc.vector.memset(eps128[:], EPS)

            y_d = dramp.tile([128, HG * T], bf16)  # yT per head at col h*T

            with tc.tile_pool(name="big", bufs=1) as bigp, \
                 tc.tile_pool(name="xtp", bufs=1) as xtp, \
                 tc.tile_pool(name="wstp", bufs=1) as wstp, \
                 tc.tile_pool(name="scp", bufs=1) as scp, \
                 tc.tile_pool(name="rowp", bufs=1) as rowp:
                q_sb = bigp.tile([128, 2 * T], bf16)   # [D, hl*T + t]
                k_sb = bigp.tile([128, 2 * T], bf16)
                v_sb = bigp.tile([128, 16 * 256], bf16)  # key tile tt at tt*256
                trigA_q = bigp.tile([128, T], bf16)
                trigB_q = bigp.tile([128, T], bf16)
                trigA_k = bigp.tile([128, T], bf16)
                trigB_k = bigp.tile([128, T], bf16)
                _main(nc, tc, xt_d, wq_d, wk_d, wv_d,
                      xtp, wstp, scp, rowp, psp,
                      womg2, b16t, freq2, gq, gqB, gk, gkB, trilA, maskB,
                      ones128, eps128, q_sb, k_sb, v_sb, y_d,
                      trigA_q, trigB_q, trigA_k, trigB_k, dbg)

            # ---- P3: out = y^T W_o (partial over heads); resident y and
            # wo column-blocks, cb-outer so the first chains start early ----
            with tc.tile_pool(name="p3w", bufs=1) as p3w, \
                 tc.tile_pool(name="p3o", bufs=1) as p3o:
                wo_slots = [None, None]

                def issue_wo(cb):
                    wob = p3w.tile([128, HG * 512], bf16, tag="wo", bufs=2,
                                   name=f"wo_{cb}")
                    for hh in range(HG):
                        nc.sync.dma_start(
                            wob[:, hh * 512:(hh + 1) * 512],
                            wo_d[hh * 128:(hh + 1) * 128,
                                 cb * 512:(cb + 1) * 512])
                    wo_slots[cb % 2] = wob

                issue_wo(0)
                yall = p3w.tile([128, HG * T], bf16, name="yall")
                for c2 in range(16):
                    nc.sync.dma_start(yall[:, c2 * 1024:(c2 + 1) * 1024],
                                      y_d[:, c2 * 1024:(c2 + 1) * 1024])
                NTI = T // 128
                for cb in range(C // 512):
                    if cb + 1 < C // 512:
                        issue_wo(cb + 1)
                    wob = wo_slots[cb % 2]
                    for ti in range(NTI):
                        ops = psp.tile([128, 512], f32, tag="y", bufs=4,
                                       name=f"ops_{cb}_{ti}")
                        for hh in range(HG):
                            nc.tensor.matmul(
                                ops[:],
                                yall[:, hh * T + ti * 128:hh * T + (ti + 1) * 128],
                                wob[:, hh * 512:(hh + 1) * 512],
                                start=(hh == 0), stop=(hh == HG - 1))
                        osb = p3o.tile([128, 512], f32, tag="osb", bufs=3)
                        nc.scalar.copy(osb[:], ops[:])
                        nc.sync.dma_start(
                            out_d[ti * 128:(ti + 1) * 128,
                                  cb * 512:(cb + 1) * 512],
                            osb[:])
    nc.compile()
    return nc


def _main(nc, tc, xt_d, wq_d, wk_d, wv_d,
          xtp, wstp, scp, rowp, psp,
          womg2, b16t, freq2, gq, gqB, gk, gkB, trilA, maskB,
          ones128, eps128, q_sb, k_sb, v_sb, y_d,
          trigA_q, trigB_q, trigA_k, trigB_k, dbg):
    f32, bf16 = dt.float32, dt.bfloat16

    # x^T tiles, half-major DMA order so P1/2a can start early
    xts = xtp.tile([128, NCT * T], bf16)  # c-tile i at cols [i*T,(i+1)*T)

    def issue_xts():
        for half in range(2):
            for i in range(NCT):
                cs = half * 1024
                nc.sync.dma_start(
                    xts[:, i * T + cs:i * T + cs + 1024],
                    xt_d[i * 128:(i + 1) * 128, cs:cs + 1024])

    sites = [(pair, wi, hl) for pair in range(4) for wi in range(2)
             for hl in range(2)]
    wp_slots = [None, None]
    wvp_slots = [None]

    def issue_panel(si):
        pair, wi, hl = sites[si]
        h = pair * 2 + hl
        w_d = (wq_d, wk_d)[wi]
        wp = wstp.tile([128, NCT * 128], bf16, tag="wp", bufs=2,
                       name=f"wp_{si}")
        for i in range(NCT):
            nc.sync.dma_start(
                wp[:, i * 128:(i + 1) * 128],
                w_d[i * 128:(i + 1) * 128, h * 128:(h + 1) * 128])
        wp_slots[si % 2] = wp

    def issue_wvp(pair):
        wvp = wstp.tile([128, NCT * 256], bf16, tag="wvp", bufs=1,
                        name=f"wvp_{pair}")
        for i in range(NCT):
            nc.sync.dma_start(
                wvp[:, i * 256:(i + 1) * 256],
                wv_d[i * 128:(i + 1) * 128, pair * 256:(pair + 1) * 256])
        wvp_slots[0] = wvp

    issue_panel(0)
    issue_wvp(0)
    issue_xts()

    # ---- P1: omega -> phi -> trig (PE does only the omega matvecs) ----
    with tc.tile_pool(name="p1p", bufs=1) as p1p:
        omega = rowp.tile([1, T], f32, tag="om")
        for J in range(NT):
            omps = psp.tile([128, 512], f32, tag="y", bufs=4,
                            name=f"omps_{J}")
            for i in range(NCT):
                nc.tensor.matmul(
                    omps[:], womg2[:, i * 128:(i + 1) * 128],
                    xts[:, i * T + J * 512:i * T + J * 512 + 512],
                    start=(i == 0), stop=(i == NCT - 1))
            nc.scalar.activation(omega[:, J * 512:(J + 1) * 512],
                                 omps[0:1, :],
                                 AF.Sigmoid, scale=1.0 / 16.0, bias=b16t[:])
        incl = rowp.tile([1, T], f32, tag="incl")
        nc.vector.tensor_tensor_scan(incl[:], omega[:], omega[:], 0.0,
                                     ALU.add, ALU.bypass)
        phi = rowp.tile([1, T], f32, tag="phi")
        nc.vector.tensor_sub(phi[:], incl[:], omega[:])
        for J in range(NT):
            sl = slice(J * 512, (J + 1) * 512)
            phi2 = p1p.tile([128, 512], f32, tag="p1", bufs=4,
                            name=f"phi2_{J}")
            nc.gpsimd.partition_broadcast(phi2[:], phi[:, sl])
            ang = p1p.tile([128, 512], f32, tag="p1", bufs=4, name=f"ang_{J}")
            nc.vector.tensor_scalar(ang[:], phi2[:], freq2[:], None,
                                    op0=ALU.mult)
            mm = p1p.tile([128, 512], f32, tag="p1", bufs=4, name=f"mm_{J}")
            nc.vector.tensor_scalar(mm[:], ang[:], INV_2PI, MAGIC,
                                    op0=ALU.mult, op1=ALU.add)
            kk = p1p.tile([128, 512], f32, tag="p1", bufs=4, name=f"kk_{J}")
            nc.vector.tensor_scalar_add(kk[:], mm[:], -MAGIC)
            red = p1p.tile([128, 512], f32, tag="p1", bufs=4, name=f"red_{J}")
            nc.vector.cody_waite_cascade(red[:], ang[:], kk[:], CW1, CW2, CW3)
            red2 = p1p.tile([128, 512], f32, tag="p1", bufs=4,
                            name=f"red2_{J}")
            nc.vector.add_range_wrap(red2[:], red[:], HALF_PI, PI, TWO_PI)
            sinr = p1p.tile([128, 512], f32, tag="p1", bufs=4,
                            name=f"sinr_{J}")
            nc.scalar.activation(sinr[:], red[:], AF.Sin)
            cosr = p1p.tile([128, 512], f32, tag="p1", bufs=4,
                            name=f"cosr_{J}")
            nc.scalar.activation(cosr[:], red2[:], AF.Sin)
            nc.scalar.activation(trigA_q[:, sl], cosr[:], AF.Copy, scale=gq[:])
            nc.scalar.activation(trigB_q[:, sl], sinr[:], AF.Copy, scale=gqB[:])
            nc.scalar.activation(trigA_k[:, sl], cosr[:], AF.Copy, scale=gk[:])
            nc.scalar.activation(trigB_k[:, sl], sinr[:], AF.Copy, scale=gkB[:])

    # ---- P2 per pair ----
    pend_norm = [None]
    pend_epi = [None]
    ssq_queue = []  # FIFO of deferred rstd tails, emitted one site later

    def flush(pend):
        if pend[0] is not None:
            pend[0]()
            pend[0] = None

    for pair in range(4):
        wvp = wvp_slots[0]

        # --- 2a: q/k for both heads ---
        for wi in range(2):
            for hl in range(2):
                si = pair * 4 + wi * 2 + hl
                if si + 1 < len(sites):
                    issue_panel(si + 1)
                wp = wp_slots[si % 2]
                trigA = (trigA_q, trigA_k)[wi]
                trigB = (trigB_q, trigB_k)[wi]
                dest = (q_sb, k_sb)[wi]
                rnbs = []
                sqs = []
                for Jp in range(2):
                    # deferred rstd tails (one-site lag, one J-group per
                    # flush so the [1,512] psum ring never backs up)
                    while len(ssq_queue) > 1:
                        ssq_queue.pop(0)()
                    qps2 = psp.tile([128, 1024], f32, tag="s", bufs=2,
                                    name=f"qps2_{si}_{Jp}")
                    for i in range(NCT):
                        for Jh in range(2):
                            J = 2 * Jp + Jh
                            nc.tensor.matmul(
                                qps2[:, Jh * 512:(Jh + 1) * 512],
                                wp[:, i * 128:(i + 1) * 128],
                                xts[:, i * T + J * 512:i * T + J * 512 + 512],
                                start=(i == 0), stop=(i == NCT - 1))
                    for Jh in range(2):
                        J = 2 * Jp + Jh
                        qps = qps2[:, Jh * 512:(Jh + 1) * 512]
                        sl = slice(J * 512, (J + 1) * 512)
                        dcol = hl * T + J * 512
                        # rotation: A + swapped-half B, gamma folded in trig
                        A = scp.tile([128, 512], f32, tag="ra", bufs=2,
                                     name=f"A_{si}_{J}")
                        nc.vector.tensor_tensor(A[:], qps, trigA[:, sl],
                                                op=ALU.mult)
                        Bt = scp.tile([128, 512], f32, tag="rb", bufs=2,
                                      name=f"Bt_{si}_{J}")
                        nc.vector.tensor_tensor(
                            Bt[0:DH, :],
                            qps2[DH:128, Jh * 512:(Jh + 1) * 512],
                            trigB[0:DH, sl], op=ALU.mult)
                        nc.vector.tensor_tensor(
                            Bt[DH:128, :],
                            qps2[0:DH, Jh * 512:(Jh + 1) * 512],
                            trigB[DH:128, sl], op=ALU.mult)
                        nc.vector.tensor_add(
                            dest[:, dcol:dcol + 512], A[:], Bt[:])
                        # sum-of-squares path (rotation preserves norms)
                        sq = scp.tile([128, 512], bf16, tag="sq", bufs=4,
                                      name=f"sq_{si}_{J}")
                        nc.scalar.activation(sq[:], qps, AF.Square)
                        sqs.append((J, sq))

                    def ssq_tail(sqs=tuple(sqs[-2:]), rnbs=rnbs, si=si):
                        for J, sq in sqs:
                            ssqps = psp.tile([128, 512], f32, tag="y", bufs=4,
                                             name=f"ssq_{si}_{J}")
                            nc.tensor.matmul(ssqps[:], ones128[:], sq[:],
                                             start=True, stop=True)
                            rnb = scp.tile([128, 512], bf16, tag="rnb",
                                           bufs=4, name=f"rnb_{si}_{J}")
                            nc.scalar.activation(rnb[:], ssqps[:],
                                                 AF.Abs_reciprocal_sqrt,
                                                 scale=1.0 / 128.0,
                                                 bias=eps128[:])
                            rnbs.append((J, rnb))
                    ssq_queue.append(ssq_tail)
                flush(pend_norm)

                def norm(dest=dest, hl=hl, rnbs=rnbs):
                    for J, rnb in rnbs:
                        dcol = hl * T + J * 512
                        nc.vector.tensor_tensor(
                            dest[:, dcol:dcol + 512],
                            dest[:, dcol:dcol + 512],
                            rnb[:], op=ALU.mult)
                pend_norm[0] = norm
        while ssq_queue:
            ssq_queue.pop(0)()
        flush(pend_norm)
        if dbg and pair == 0:
            nc.sync.dma_start(dbg["q"][:], q_sb[:])
            nc.sync.dma_start(dbg["k"][:], k_sb[:])

        # --- 2b: v for both heads; each accumulation chain owns a full PSUM
        # bank: quarters 0 and 2 of two [128,1024] tiles = 4 banks ---
        for tq in range(4):
            vps = []
            for q4 in range(2):
                vps.append(psp.tile([128, 1024], f32, tag="s", bufs=2,
                                    name=f"vps_{pair}_{tq}_{q4}"))
            for i in range(NCT):
                for t in range(4):
                    tt = tq * 4 + t
                    nc.tensor.matmul(
                        vps[t // 2][:, (t % 2) * 512:(t % 2) * 512 + 256],
                        xts[:, i * T + tt * 128:i * T + (tt + 1) * 128],
                        wvp[:, i * 256:(i + 1) * 256],
                        start=(i == 0), stop=(i == NCT - 1))
            for t in range(4):
                tt = tq * 4 + t
                nc.vector.tensor_copy(
                    v_sb[:, tt * 256:(tt + 1) * 256],
                    vps[t // 2][:, (t % 2) * 512:(t % 2) * 512 + 256])
        if pair + 1 < 4:
            issue_wvp(pair + 1)
        if dbg and pair == 0:
            nc.sync.dma_start(dbg["v"][:], v_sb[:])

        # --- 2c: attention, software-pipelined lookahead-2 ---
        for hl in range(2):
            h = pair * 2 + hl
            for J in range(NT):
                nI = 4 * J + 4
                yps = psp.tile([128, 512], f32, tag="y", bufs=4,
                               name=f"yps_{pair}_{hl}_{J}")
                dps = psp.tile([128, 512], f32, tag="y", bufs=4,
                               name=f"dps_{pair}_{hl}_{J}")
                exq = []

                def consume(ex2, I0, yps=yps, dps=dps, hl=hl, nI=nI):
                    for half2 in range(2):
                        I = I0 + half2
                        exsl = ex2[:, half2 * 512:(half2 + 1) * 512]
                        nc.tensor.matmul(
                            yps[:],
                            v_sb[:, I * 256 + hl * 128:I * 256 + hl * 128 + 128],
                            exsl, start=(I == 0), stop=(I == nI - 1))
                        nc.tensor.matmul(
                            dps[:], ones128[:], exsl,
                            start=(I == 0), stop=(I == nI - 1))

                for Ip in range(nI // 2):
                    sps2 = psp.tile([128, 1024], f32, tag="s", bufs=2,
                                    name=f"sps_{pair}_{hl}_{J}_{Ip}")
                    for half2 in range(2):
                        I = 2 * Ip + half2
                        diag = I >= 4 * J
                        osl = sps2[:, half2 * 512:(half2 + 1) * 512]
                        nc.tensor.matmul(
                            osl,
                            k_sb[:, hl * T + I * 128:hl * T + (I + 1) * 128],
                            q_sb[:, hl * T + J * 512:hl * T + (J + 1) * 512],
                            start=True, stop=(not diag))
                        if diag:
                            r = I - 4 * J
                            nc.tensor.matmul(
                                osl, trilA[:], maskB[:, r * 512:(r + 1) * 512],
                                start=False, stop=True)
                    ex2 = scp.tile([128, 1024], bf16, tag="ex", bufs=3,
                                   name=f"ex_{pair}_{hl}_{J}_{Ip}")
                    nc.scalar.activation(ex2[:], sps2[:], AF.Exp, scale=SCL)
                    exq.append((ex2, 2 * Ip))
                    if len(exq) > 2:
                        consume(*exq.pop(0))
                    if Ip == 0:
                        flush(pend_epi)
                while exq:
                    consume(*exq.pop(0))

                def epilogue(yps=yps, dps=dps, h=h, J=J):
                    rb = scp.tile([128, 512], f32, tag="rbc", bufs=2,
                                  name=f"rb_{h}_{J}")
                    nc.vector.reciprocal_approx_fast(out=rb[:], in_=dps[:])
                    yt = scp.tile([128, 512], bf16, tag="yt", bufs=2,
                                  name=f"yt_{h}_{J}")
                    nc.vector.tensor_tensor(yt[:], yps[:], rb[:], op=ALU.mult)
                    nc.sync.dma_start(
                        y_d[:, h * T + J * 512:h * T + (J + 1) * 512], yt[:])
                    if dbg:
                        nc.sync.dma_start(
                            dbg["y"][:, h * T + J * 512:h * T + (J + 1) * 512],
                            yt[:])
                pend_epi[0] = epilogue
        flush(pend_epi)


def _host_prep(inputs):
    bf = ml_dtypes.bfloat16
    x = np.asarray(inputs["x"], dtype=np.float32)
    Wq = np.asarray(inputs["Wq"], dtype=np.float32)
    Wk = np.asarray(inputs["Wk"], dtype=np.float32)
    Wv = np.asarray(inputs["Wv"], dtype=np.float32)
    Wo = np.asarray(inputs["Wo"], dtype=np.float32)
    w_omega = np.asarray(inputs["w_omega"], dtype=np.float32)
    b_omega = np.asarray(inputs["b_omega"], dtype=np.float32)
    log_freq = np.asarray(inputs["log_freq"], dtype=np.float32)
    q_gamma = np.asarray(inputs["q_gamma"], dtype=np.float32)
    k_gamma = np.asarray(inputs["k_gamma"], dtype=np.float32)

    womg = w_omega.reshape(NCT, 128).T.astype(np.float32)
    # replicated across output rows: womg2[:, i*128+c] = w_omega[i*128+:] col c
    womg2 = np.repeat(womg.T[:, :, None], 128, axis=2)  # [i, 128k, 128c]
    womg2 = womg2.transpose(1, 0, 2).reshape(128, NCT * 128).astype(bf)
    b16 = (b_omega / 16.0).reshape(1, 1).astype(np.float32)
    logf2 = np.concatenate([log_freq, log_freq]).reshape(128, 1)
    gqv = q_gamma.reshape(128, 1).astype(np.float32)
    gqB = np.concatenate([q_gamma[:DH], -q_gamma[DH:]]).reshape(128, 1)
    gkv = k_gamma.reshape(128, 1).astype(np.float32)
    gkB = np.concatenate([k_gamma[:DH], -k_gamma[DH:]]).reshape(128, 1)
    kk = np.arange(128)
    trilA = (kk[:, None] <= kk[None, :]).astype(bf)  # [k, p] = (k <= p)
    p = np.arange(128)[:, None]
    c = np.arange(512)[None, :]
    maskB = np.concatenate(
        [(NEG * ((p + r * 128) > c)).astype(np.float32) for r in range(4)],
        axis=1).astype(bf)
    ones128 = np.ones((128, 128), dtype=bf)

    in_maps = []
    for core in range(8):
        b, g = core // 2, core % 2
        in_maps.append({
            "xt": np.ascontiguousarray(x[b].T).astype(bf),
            "wq": np.ascontiguousarray(Wq[g * GD:(g + 1) * GD, :].T).astype(bf),
            "wk": np.ascontiguousarray(Wk[g * GD:(g + 1) * GD, :].T).astype(bf),
            "wv": np.ascontiguousarray(Wv[g * GD:(g + 1) * GD, :].T).astype(bf),
            "wo": np.ascontiguousarray(Wo[:, g * GD:(g + 1) * GD].T).astype(bf),
            "womg2": womg2, "b16": b16,
            "logf2": logf2.astype(np.float32),
            "gq": gqv, "gqB": gqB.astype(np.float32),
            "gk": gkv, "gkB": gkB.astype(np.float32),
            "trilA": trilA, "maskB": maskB, "ones128": ones128,
        })
    return in_maps


def kernel(**inputs) -> np.ndarray:
    if "nc" not in _CACHE:
        _CACHE["nc"] = _build()
    nc = _CACHE["nc"]
    in_maps = _host_prep(inputs)
    res = run_bass_kernel_spmd(nc, in_maps, core_ids=list(range(8)))
    out = np.empty((B, T, C), dtype=np.float32)
    for b in range(B):
        out[b] = res.results[2 * b]["out"] + res.results[2 * b + 1]["out"]
    return out
